# revision 1
# baseline (speedup 1.0000x reference)
import sys

sys.path.insert(0, "/opt/trn_rl_repo")

import numpy as np

import concourse.bacc as bacc
import concourse.mybir as mybir
import concourse.tile as tile
from concourse.bass_utils import run_bass_kernel_spmd
from concourse.masks import make_identity

# Problem constants (nn_AMMConv2d: 3x3 conv via product quantization, STE forward)
NC, K, SUB = 16, 16, 72
CIN, COUT = 128, 256
H = W = 56
B = 8
L = H * W              # 3136 positions per image
PW = W + 2             # padded width 58
NT = 128               # stationary span per tile (128 cols -> FWL; junk rows skipped)
NTV = 112              # valid positions per tile (2 output rows)
NTILES = L // NTV      # 28
XPL = PW * PW + 14     # xp with 1 guard col front, 13 back
CK = NC * K            # 256
CS = NC * SUB          # 1152 = 9 * 128 block-diag contraction rows

F32 = mybir.dt.float32
F32R = mybir.dt.float32r
BF16 = mybir.dt.bfloat16


def build():
    nc = bacc.Bacc("TRN2", target_bir_lowering=False, debug=False)

    xh_ext = nc.declare_dram_parameter("xpad_hi", [CIN, XPL], BF16, isOutput=False)
    xl_ext = nc.declare_dram_parameter("xpad_lo", [CIN, XPL], BF16, isOutput=False)
    chl_ext = nc.declare_dram_parameter("cmm_hl", [CIN, 9 * 2 * CK], BF16, isOutput=False)
    cbd_ext = nc.declare_dram_parameter("cent_bd", [CIN, 9 * CK], BF16, isOutput=False)
    wbd_ext = nc.declare_dram_parameter("w_bd", [CIN, 9 * COUT], BF16, isOutput=False)
    bias_ext = nc.declare_dram_parameter("bias_bc", [CIN, COUT], F32, isOutput=False)
    out_ext = nc.declare_dram_parameter("out", [L, COUT], F32, isOutput=True)

    with tile.TileContext(nc) as tc:
        with (
            tc.tile_pool(name="const", bufs=1) as const_pool,
            tc.tile_pool(name="cpsum", bufs=1, space="PSUM") as cpsum_pool,
            tc.tile_pool(name="work", bufs=4) as work,
            tc.tile_pool(name="spsum", bufs=3, space="PSUM") as spsum,
            tc.tile_pool(name="tpsum", bufs=2, space="PSUM") as tpsum,
            tc.tile_pool(name="opsum", bufs=2, space="PSUM") as opsum,
        ):
            # ---------- prologue: load constants ----------
            xp_hi = const_pool.tile([CIN, XPL], BF16)
            nc.sync.dma_start(xp_hi[:], xh_ext[:])
            xp_lo = const_pool.tile([CIN, XPL], BF16)
            nc.sync.dma_start(xp_lo[:], xl_ext[:])
            cents_hl = const_pool.tile([CIN, 9, 2 * CK], BF16)
            nc.sync.dma_start(
                cents_hl[:].rearrange("p t c -> p (t c)"), chl_ext[:]
            )
            cbd = const_pool.tile([CIN, 9, CK], BF16)  # block-diag (c,s) x (c,k)
            nc.sync.dma_start(cbd[:].rearrange("p t c -> p (t c)"), cbd_ext[:])
            wbd = const_pool.tile([CIN, 9, COUT], BF16)  # w rows in (c,s) order
            nc.sync.dma_start(wbd[:].rearrange("p t c -> p (t c)"), wbd_ext[:])
            bias_bc = const_pool.tile([CIN, COUT], F32)
            nc.sync.dma_start(bias_bc[:], bias_ext[:])

            ones_bf = const_pool.tile([CIN, CIN], F32)
            nc.vector.memset(ones_bf[:], 1.0)
            ones = const_pool.tile([CIN, CIN], BF16)
            nc.vector.tensor_copy(ones[:], ones_bf[:])
            ident = const_pool.tile([NT, NT], BF16)
            make_identity(nc, ident[:])

            # ---------- prologue: c2[ck] broadcast over partitions ----------
            cfull = const_pool.tile([CIN, 9 * CK], F32)
            nc.vector.tensor_tensor(
                cfull[:].rearrange("p (t c) -> p t c", t=9),
                cents_hl[:, :, :CK],
                cents_hl[:, :, CK:],
                mybir.AluOpType.add,
            )
            sq = const_pool.tile([CIN, 9 * CK], F32)
            nc.vector.tensor_mul(sq[:], cfull[:], cfull[:])
            sq_hi = const_pool.tile([CIN, 9, CK], BF16)
            nc.vector.tensor_copy(sq_hi[:].rearrange("p t c -> p (t c)"), sq[:])
            sq_lo = const_pool.tile([CIN, 9, CK], BF16)
            nc.vector.tensor_tensor(
                sq_lo[:].rearrange("p t c -> p (t c)"),
                sq[:],
                sq_hi[:].rearrange("p t c -> p (t c)"),
                mybir.AluOpType.subtract,
            )
            c2_sb = const_pool.tile([CIN, CK], F32)
            lut_sb = [
                const_pool.tile([CIN, COUT], BF16, tag=f"lut{j}", name=f"lut{j}") for j in range(2)
            ]

            def emit_c2_lut():
                c2_ps = tpsum.tile([CIN, CK], F32, tag="lutc2", name="c2_ps", bufs=1)
                for t in range(9):
                    for i, sqt in enumerate([sq_hi, sq_lo]):
                        nc.tensor.matmul(
                            c2_ps[:],
                            ones[:],
                            sqt[:, t, :].opt(),
                            start=(t == 0 and i == 0),
                            stop=(t == 8 and i == 1),
                        )
                nc.scalar.activation(
                    c2_sb[:], c2_ps[:], mybir.ActivationFunctionType.Copy
                )
                # lut[ck, o] = cent @ weight + bias/16; chunk q covers rows
                # [128q, 128q+128) of (c,s); ck-half j covers rows [576j, 576j+576)
                for j in range(2):
                    lut_ps = tpsum.tile([CIN, CK], F32, tag="lutc2", name="lut_ps", bufs=1)
                    qs = [q for q in range(9)
                          if q * 128 < (j + 1) * 576 and (q + 1) * 128 > j * 576]
                    for idx, q in enumerate(qs):
                        nc.tensor.matmul(
                            lut_ps[:, :],
                            cbd[:, q, j * CIN : (j + 1) * CIN].opt(),
                            wbd[:, q, :].opt(),
                            start=(idx == 0),
                            stop=(idx == len(qs) - 1),
                        )
                    nc.vector.scalar_tensor_tensor(
                        lut_sb[j][:],
                        bias_bc[:],
                        1.0 / 16.0,
                        lut_ps[:],
                        op0=mybir.AluOpType.mult,
                        op1=mybir.AluOpType.add,
                    )

            # ---------- main loop: scores lead epilogues by one tile ----------
            def emit_scores(t):
                oh0 = 2 * t
                base = 1 + (oh0 + 1) * PW  # guard col + center-row start
                s_ps = spsum.tile([CIN, 2 * CK], F32, tag="scores", name="s_ps")
                for kk in range(9):
                    kh, kw = divmod(kk, 3)
                    off = base + (kh - 1) * PW + (kw - 1)
                    nc.tensor.matmul(
                        s_ps[:NT, :],
                        xp_hi[:, off : off + NT],
                        cents_hl[:, kk, :].opt(),
                        start=(kk == 0),
                        stop=False,
                    )
                    nc.tensor.matmul(
                        s_ps[:NT, :CK],
                        xp_lo[:, off : off + NT],
                        cents_hl[:, kk, :CK].opt(),
                        start=False,
                        stop=(kk == 8),
                    )
                return s_ps

            def emit_epilogue(t, s_ps):
                # g = xc - c2/2 ; argmax over k within each codebook
                s_sb = work.tile([NT, 2 * CK], F32, tag="ssb", name="s_sb")
                nc.scalar.activation(
                    s_sb[:], s_ps[:NT, :], mybir.ActivationFunctionType.Copy
                )
                g1 = work.tile([NT, CK], F32, tag="g1", name="g1")
                nc.vector.scalar_tensor_tensor(
                    g1[:],
                    c2_sb[:NT, :],
                    -0.5,
                    s_sb[:, :CK],
                    op0=mybir.AluOpType.mult,
                    op1=mybir.AluOpType.add,
                )
                g = work.tile([NT, CK], F32, tag="g", name="g")
                nc.vector.tensor_tensor(
                    g[:], g1[:], s_sb[:, CK:], mybir.AluOpType.add
                )
                g3 = g[:].rearrange("p (c k) -> p c k", c=NC)
                gmax = work.tile([NT, NC], F32, tag="gmax", name="gmax")
                nc.vector.tensor_reduce(
                    gmax[:], g3, axis=mybir.AxisListType.X, op=mybir.AluOpType.max
                )
                mask = work.tile([NT, CK], BF16, tag="mask", name="mask")
                nc.vector.tensor_tensor(
                    mask[:].rearrange("p (c k) -> p c k", c=NC),
                    g3,
                    gmax[:].unsqueeze(2).broadcast_to([NT, NC, K]),
                    mybir.AluOpType.is_equal,
                )

                o_ps = opsum.tile([NT, COUT], F32, tag="out", name="o_ps")
                mt_ps = tpsum.tile([CIN, 2 * NT], BF16, tag="mt", name="mt_ps")
                for j in range(2):
                    nc.tensor.transpose(
                        mt_ps[:, j * NT : (j + 1) * NT],
                        mask[:, j * CIN : (j + 1) * CIN],
                        ident[:],
                    )
                oh_sb = work.tile([CIN, 2 * NT], BF16, tag="oh", name="oh_sb")
                nc.scalar.activation(
                    oh_sb[:], mt_ps[:], mybir.ActivationFunctionType.Copy
                )
                for j in range(2):
                    nc.tensor.matmul(
                        o_ps[:],
                        oh_sb[:, j * NT : (j + 1) * NT],
                        lut_sb[j][:],
                        start=(j == 0),
                        stop=(j == 1),
                    )

                o_sb = work.tile([NT, COUT], F32, tag="osb", name="o_sb")
                nc.scalar.activation(
                    o_sb[:], o_ps[:], mybir.ActivationFunctionType.Copy
                )
                nc.sync.dma_start(
                    out_ext[t * NTV : t * NTV + W, :], o_sb[1 : W + 1, :]
                )
                nc.sync.dma_start(
                    out_ext[t * NTV + W : (t + 1) * NTV, :],
                    o_sb[PW + 1 : PW + 1 + W, :],
                )

            pend = []
            for t in range(NTILES):
                pend.append((t, emit_scores(t)))
                if t == 1:
                    emit_c2_lut()
                if t >= 1:
                    emit_epilogue(*pend.pop(0))
            for item in pend:
                emit_epilogue(*item)

    nc.compile()
    return nc


def _bf16():
    import ml_dtypes

    return ml_dtypes.bfloat16


def _bf16_split(a):
    hi = a.astype(_bf16())
    lo = (a - hi.astype(np.float32)).astype(_bf16())
    return hi, lo


def _pack_hl(a):
    # [CIN, 9*CK] f32 -> [CIN, 9*(2*CK)] bf16 packed (hi | lo) per tap
    hi, lo = _bf16_split(a.reshape(CIN, 9, CK))
    out = np.concatenate([hi, lo], axis=2)
    return np.ascontiguousarray(out.reshape(CIN, 9 * 2 * CK))


def _pad_x(xi):
    xp = np.zeros((CIN, XPL), dtype=np.float32)
    xp[:, 1 : 1 + PW * PW] = np.pad(xi, ((0, 0), (1, 1), (1, 1))).reshape(
        CIN, PW * PW
    )
    return _bf16_split(xp)


def _prep_consts(centroids, weight):
    # cents_mm[ci, kk, c*16+k] = centroids[c, k, (ci%8)*9+kk] for c = ci//8, else 0
    cents_mm = np.zeros((9, CIN, CK), dtype=np.float32)
    cs = centroids.reshape(NC, K, 8, 9)  # s = a*9 + kk
    for c in range(NC):
        for a in range(8):
            cents_mm[:, 8 * c + a, c * K : (c + 1) * K] = cs[c, :, a, :].T
    cents_mm = np.ascontiguousarray(
        cents_mm.transpose(1, 0, 2).reshape(CIN, 9 * CK)
    )

    # Block-diag over (c, s) rows: g = q*128 + r = c*SUB + s
    # cent_bd[r, q, ck] = centroids[c, k, s] if ck // K == c else 0
    # w_bd[r, q, o] = weight[c, s, o]
    cent_bd = np.zeros((CIN, 9, CK), dtype=np.float32)
    w_bd = np.zeros((CIN, 9, COUT), dtype=np.float32)
    for q in range(9):
        for r in range(CIN):
            g = q * CIN + r
            c, s = divmod(g, SUB)
            cent_bd[r, q, c * K : (c + 1) * K] = centroids[c, :, s]
            w_bd[r, q, :] = weight[c, s, :]
    cent_bd = np.ascontiguousarray(cent_bd.reshape(CIN, 9 * CK))
    w_bd = np.ascontiguousarray(w_bd.reshape(CIN, 9 * COUT))
    return cents_mm, cent_bd, w_bd


_NC_CACHE = []


def kernel(x, centroids, weight, inverse_temperature_logit, bias):
    x = np.asarray(x, dtype=np.float32)
    centroids = np.asarray(centroids, dtype=np.float32)
    weight = np.asarray(weight, dtype=np.float32)
    bias = np.asarray(bias, dtype=np.float32)

    if not _NC_CACHE:
        _NC_CACHE.append(build())
    nc = _NC_CACHE[0]

    cents_mm, cent_bd, w_bd = _prep_consts(centroids, weight)
    cmm_hl = _pack_hl(cents_mm)
    bias_bc = np.ascontiguousarray(np.broadcast_to(bias, (CIN, COUT)))
    xs = [_pad_x(x[i]) for i in range(B)]
    in_maps = [
        {
            "xpad_hi": xs[i][0],
            "xpad_lo": xs[i][1],
            "cmm_hl": cmm_hl,
            "cent_bd": cent_bd.astype(_bf16()),
            "w_bd": w_bd.astype(_bf16()),
            "bias_bc": bias_bc,
        }
        for i in range(B)
    ]
    res = run_bass_kernel_spmd(nc, in_maps, core_ids=list(range(B)))
    out = np.stack([res.results[i]["out"] for i in range(B)])  # [8, L, COUT]
    out = out.reshape(B, H, W, COUT).transpose(0, 3, 1, 2)
    return np.ascontiguousarray(out.astype(np.float32))

